# revision 39
# baseline (speedup 1.0000x reference)
"""Gaussian falloff vortex-velocity kernel for Trainium2 (Bass/Tile).

Math per batch element b (single vortex y,x,tau,sig per batch):
    d1 = py - y;  d2 = px - x;  q = d1^2 + d2^2
    s  = tau * exp(-q/sig^2) / sqrt(q)
    out[..., 0] = s * d2;  out[..., 1] = -s * d1

16-bit formulation. The host quantizes the points with a per-batch
AFFINE map whose zero-point is the vortex and whose scale is
g = sqrt(2)/sig:  t1 = (y - py)*g,  t2 = (px - x)*g, rounded to bf16.
The zero-point kills catastrophic cancellation near the vortex (the
quantization error is relative to the distance d), and the scale
makes the on-chip math scalar-free:

    q' = t1^2 + t2^2  = 2 q / sig^2
    z' = q' + Ln(q' + 1e-30)
    s' = Exp(-z'/2 + ln tau)  = tau * exp(-q/sig^2) * sig/sqrt(2 q)
    out = s' * (t1, t2)       -- the g factors cancel exactly, giving
                                 the true velocity (v = s*d is
                                 invariant under scaling of d).

The eps in the Ln keeps s' finite at q'==0 so out = 0 * s' = 0; fp16
overflow of z' (q' > 65504 for tiny sig) gives s' = exp(-inf) = 0,
which matches the true falloff there. Outputs are fp16 (|v| <= tau
< 1), upcast on host. Simulated error vs the f32 reference:
l2 = 3.0e-3, scale-relative absmax = 7.9e-3 (gate: 2e-2).

Per core: 8 batches, each point-plane [512,512] -> [128, 2048].
Engine assignment (GpSimd deliberately unused -- it shares an SBUF
port with the vector engine and throttles concurrent DVE tensor ops
~4x, measured; Pool only drives the store-DMA descriptor ring, which
does not touch the SBUF data path):

    Qe = t1*t1            DVE tt bf16 (2x 16-bit mode, ~0.59ns/col)
    Qo = t2*t2            ACT Square (1x, any dtype)
    q' = Qe + Qo          DVE tt
    L  = Ln(q'+eps)       ACT fp16
    z' = q' + L           DVE tt fp16
    s' = Exp(-z'/2+lnt)   ACT bf16, per-batch bias AP
    O  = T * [s'|s']      DVE tt, ONE merged 4096-col op: s' rides a
                          stride-0 broadcast AP; plane order follows T
                          ([t1|t2]) so O = [v1|v0], unpacked swapped on
                          the host. (Merged 4096-col tts hold the 2x
                          mode; an earlier 1x reading was GpSimd
                          contention, not a size limit.)

Per batch: DVE 1223+1217+1217+2284 = 5.94us, ACT 3*1989 = 5.97us --
balanced. DMA 16MB/core at ~400GB/s = 40us, fully hidden. The 8-deep
software pipeline gives every cross-engine edge >= 1 full step of
slack, so no engine head-of-line blocks (each stage's inputs finished
during the previous step).
"""

import numpy as np
import ml_dtypes

import concourse.bass as bass
import concourse.bacc as bacc
import concourse.mybir as mybir
from concourse.tile import TileContext
from concourse.bass_utils import run_bass_kernel_spmd
from concourse.hw_specs import get_activation_tables

N_CORES = 8
B_PER_CORE = 8          # 64 batches / 8 cores
P = 128                 # SBUF partitions
W = 2048                # per-plane columns per partition (512*512/128)

BF16 = ml_dtypes.bfloat16

_PROGRAM = None


def _pin_act_table_set(arch: str):
    """Make all our activation functions resolve to the single
    `natural_log_exp_and_others` table set. The table-load inserter picks
    the FIRST set containing each function (Exp -> exp_and_others,
    Ln -> natural_log), which thrashes 2 table loads (~2.6us) per batch.
    get_activation_tables() is functools.cached and returns a mutable
    dict of sets; removing our functions from every other set (keeping
    indices intact) makes the combined set the unique first match."""
    AF = mybir.ActivationFunctionType
    try:
        tables = get_activation_tables(arch)
        keep = "natural_log_exp_and_others"
        needed = {AF.Identity, AF.Ln, AF.Exp, AF.Square, AF.Copy}
        if keep not in tables or not needed <= tables[keep]:
            return  # unexpected table layout: skip pinning (correct, slower)
        for name, fns in tables.items():
            if name != keep:
                fns -= needed
    except Exception:
        pass


def _build_program():
    f32 = mybir.dt.float32
    f16 = mybir.dt.float16
    bf16 = mybir.dt.bfloat16
    AF = mybir.ActivationFunctionType
    OP = mybir.AluOpType

    nc = bacc.Bacc(
        "TRN2",
        target_bir_lowering=False,
        debug=False,
        num_devices=N_CORES,
    )
    _pin_act_table_set(nc.m.arch)
    # tin cols: [t1-plane | t2-plane]; tout cols: [v1-plane | v0-plane]
    tin = nc.declare_dram_parameter("tin", [B_PER_CORE * P, 2 * W], bf16, isOutput=False)
    # consts: ln(tau) per batch + a trailing eps (1e-30) column for Ln's bias
    cst = nc.declare_dram_parameter("consts", [P, B_PER_CORE + 1], f32, isOutput=False)
    out = nc.declare_dram_parameter("tout", [B_PER_CORE * P, 2 * W], f16, isOutput=True)

    with TileContext(nc) as tc:
        with (
            tc.tile_pool(name="cpool", bufs=1) as cpool,
            tc.tile_pool(name="tp", bufs=8) as tp,        # T tiles (in planes)
            tc.tile_pool(name="qb", bufs=3) as qb_pool,   # squares
            tc.tile_pool(name="qq", bufs=4) as qq_pool,   # q'
            tc.tile_pool(name="lp", bufs=3) as l_pool,    # L
            tc.tile_pool(name="zp", bufs=3) as z_pool,    # z'
            tc.tile_pool(name="sp", bufs=3) as s_pool,    # s'
            tc.tile_pool(name="op", bufs=3) as o_pool,    # out planes
        ):
            # Consts ride the gpsimd DGE ring so the first T load is the
            # very first transfer on the sync ring (consts aren't needed
            # until the first Ln, ~4 steps in).
            c = cpool.tile([P, B_PER_CORE + 1], f32)
            nc.gpsimd.dma_start(c[:], cst[:])
            eps_ap = c[:, B_PER_CORE : B_PER_CORE + 1]

            # No warm-up activation: walrus hangs the ACT table load on the
            # first Square, which executes during that op's DMA wait -- off
            # the critical path for free.

            def cap(b):
                return c[:, b : b + 1]

            # 8-stage software pipeline over work items (batch col-chunks):
            #   A (step i):   load T(i)                       [SP ring]
            #   B (step i+1): Qe = t1^2 (DVE); Qo = t2^2 (ACT Square)
            #   C (step i+2): q' = Qe + Qo                    [DVE]
            #   D (step i+3): L = Ln(q'+eps)                  [ACT]
            #   E (step i+4): z' = q' + L                     [DVE]
            #   F (step i+5): s' = Exp(-z'/2 + ln tau)        [ACT]
            #   G (step i+6): O = T * [s'|s']  (merged)       [DVE]
            #   H (step i+7): store O                         [Pool DGE ring]
            # Every cross-engine dependency is at least one full step old, so
            # no engine's stream ever head-of-line blocks on in-flight work.
            # First/last batches split in col-halves to shorten fill/drain.
            items = []
            for b in range(B_PER_CORE):
                if b == 0:
                    # Graduated lead-in: a small first item gets the pipeline
                    # computing as soon as possible after DMA spin-up.
                    items.append((b, 0, W // 8))
                    items.append((b, W // 8, 3 * W // 8))
                    items.append((b, W // 2, W // 2))
                elif b == B_PER_CORE - 1:
                    items.append((b, 0, W // 2))
                    items.append((b, W // 2, W // 2))
                else:
                    items.append((b, 0, W))
            Ts, Qbs, qs, Ls, zs, ss, Os = {}, {}, {}, {}, {}, {}, {}

            def stage_a(i):
                b, c0, w = items[i]
                rows = slice(b * P, (b + 1) * P)
                T = tp.tile([P, 2 * w], bf16, tag="T")
                if w == W:
                    nc.sync.dma_start(T[:], tin[rows, :])
                else:
                    nc.sync.dma_start(T[:, :w], tin[rows, c0 : c0 + w])
                    nc.sync.dma_start(T[:, w:], tin[rows, W + c0 : W + c0 + w])
                Ts[i] = T

            def stage_b(i):
                b, c0, w = items[i]
                T = Ts[i]
                # Measured balance point: DVE square = t1 plane, ACT Square =
                # t2 plane puts both engines at ~5.95us/item. The DVE stream
                # order (z' last, behind the 2.3us O-product) already gives
                # the Ln->z' edge its slack, so no extra ACT-ahead margin.
                k = 0
                Qb = qb_pool.tile([P, 2 * w], bf16, tag="Qb")
                nc.vector.tensor_tensor(Qb[:, : w + k], T[:, : w + k], T[:, : w + k], OP.mult)
                nc.scalar.activation(Qb[:, w + k :], T[:, w + k :], AF.Square)
                Qbs[i] = Qb

            def stage_c(i):
                b, c0, w = items[i]
                Qb = Qbs[i]
                q = qq_pool.tile([P, w], bf16, tag="q")
                nc.vector.tensor_tensor(q[:], Qb[:, :w], Qb[:, w:], OP.add)
                qs[i] = q

            def stage_d(i):
                b, c0, w = items[i]
                L = l_pool.tile([P, w], f16, tag="L")
                nc.scalar.activation(L[:], qs[i][:], AF.Ln, bias=eps_ap)
                Ls[i] = L

            def stage_e(i):
                b, c0, w = items[i]
                z = z_pool.tile([P, w], f16, tag="z")
                nc.vector.tensor_tensor(z[:], qs[i][:], Ls[i][:], OP.add)
                zs[i] = z

            def stage_f(i):
                b, c0, w = items[i]
                s = s_pool.tile([P, w], bf16, tag="s")
                nc.scalar.activation(s[:], zs[i][:], AF.Exp, bias=cap(b), scale=-0.5)
                ss[i] = s

            def stage_g(i):
                b, c0, w = items[i]
                T, s = Ts[i], ss[i]
                O = o_pool.tile([P, 2 * w], f16, tag="O")
                Ov = O.rearrange("p (c w) -> p c w", c=2)
                Tv = T.rearrange("p (c w) -> p c w", c=2)
                s_bc = s.rearrange("p (c w) -> p c w", c=1)[:].to_broadcast((P, 2, w))
                nc.vector.tensor_tensor(Ov[:], Tv[:], s_bc, OP.mult)
                Os[i] = O
                del Ts[i], Qbs[i], qs[i], Ls[i], zs[i], ss[i]

            def stage_h(i):
                b, c0, w = items[i]
                rows = slice(b * P, (b + 1) * P)
                O = Os[i]
                # Store triggers ride the GpSimd DGE ring: descriptor
                # generation does not touch the SBUF data path (unlike
                # gpsimd COMPUTE, which contends with DVE), and a separate
                # ring keeps store triggers from delaying loads.
                if w == W:
                    nc.gpsimd.dma_start(out[rows, :], O[:])
                else:
                    nc.gpsimd.dma_start(out[rows, c0 : c0 + w], O[:, :w])
                    nc.gpsimd.dma_start(out[rows, W + c0 : W + c0 + w], O[:, w:])
                del Os[i]

            # Within each step, the DVE stream runs [Qb, q', O, z']: the
            # O-product's input (s', 2 steps older) is safer than z's L, so
            # emitting O before z' gives the Ln->z' edge maximal slack.
            NI = len(items)
            for t in range(NI + 7):
                if t < NI:
                    stage_a(t)
                if 1 <= t < NI + 1:
                    stage_b(t - 1)
                if 2 <= t < NI + 2:
                    stage_c(t - 2)
                if 3 <= t < NI + 3:
                    stage_d(t - 3)
                if 6 <= t < NI + 6:
                    stage_g(t - 6)
                if 4 <= t < NI + 4:
                    stage_e(t - 4)
                if 5 <= t < NI + 5:
                    stage_f(t - 5)
                if t >= 7:
                    stage_h(t - 7)

    nc.compile()
    return nc


def _get_program():
    global _PROGRAM
    if _PROGRAM is None:
        _PROGRAM = _build_program()
    return _PROGRAM


def _make_in_maps(vortex_feature, points):
    B = points.shape[0]
    vf = np.asarray(vortex_feature, dtype=np.float32).reshape(B, 6)
    y, x, tau, sig = vf[:, 0], vf[:, 1], vf[:, 2], vf[:, 3]
    # Affine quantization scale g = sqrt(2)/sig. The sig floor keeps g (and
    # t*g) finite; for sig that small the true falloff underflows to 0, and
    # on-chip z' overflows fp16 -> s' = exp(-inf) = 0, matching it.
    g = (np.sqrt(2.0, dtype=np.float32) / np.maximum(sig, 1e-6)).astype(np.float32)
    with np.errstate(divide="ignore"):
        lnt = np.log(tau).astype(np.float32)  # tau==0 -> -inf -> s'=0
    ncol = B_PER_CORE + 1

    pts = np.asarray(points, dtype=np.float32)
    # Zero-point at the vortex (no cancellation near it), then scale by g.
    # t1 is negated (y - py) so out[...,1] = s' * t1.
    t1 = ((y[:, None, None] - pts[..., 0]) * g[:, None, None]).astype(BF16)
    t2 = ((pts[..., 1] - x[:, None, None]) * g[:, None, None]).astype(BF16)

    in_maps = []
    for i in range(N_CORES):
        sl = slice(i * B_PER_CORE, (i + 1) * B_PER_CORE)
        tin = np.concatenate(
            [t1[sl].reshape(B_PER_CORE * P, W), t2[sl].reshape(B_PER_CORE * P, W)],
            axis=1,
        )
        crow = np.concatenate([lnt[sl], np.float32([1e-30])]).reshape(1, ncol)
        cshard = np.ascontiguousarray(np.broadcast_to(crow, (P, ncol)))
        in_maps.append({"tin": np.ascontiguousarray(tin), "consts": cshard})
    return in_maps


def run(vortex_feature, points, trace=False, tmpdir=None):
    nc = _get_program()
    in_maps = _make_in_maps(vortex_feature, points)
    # The first execution of a freshly-loaded NEFF occasionally hits a
    # transient NRT_EXEC_UNIT_UNRECOVERABLE; a retry reliably succeeds.
    last_err = None
    for _ in range(3):
        try:
            res = run_bass_kernel_spmd(nc, in_maps, list(range(N_CORES)), trace=trace, tmpdir=tmpdir)
            break
        except Exception as err:  # noqa: BLE001
            last_err = err
    else:
        raise last_err
    B, H, Wd, _ = points.shape
    out = np.empty((B, H, Wd, 2), dtype=np.float32)
    for i in range(N_CORES):
        sl = slice(i * B_PER_CORE, (i + 1) * B_PER_CORE)
        o = res.results[i]["tout"].astype(np.float32)
        # Device plane order follows tin ([t1|t2]), so plane 0 = s'*t1 =
        # out[...,1] and plane 1 = s'*t2 = out[...,0].
        out[sl, ..., 1] = o[:, :W].reshape(B_PER_CORE, H, Wd)
        out[sl, ..., 0] = o[:, W:].reshape(B_PER_CORE, H, Wd)
    return out, res


def kernel(vortex_feature: np.ndarray, points: np.ndarray) -> np.ndarray:
    out, _ = run(vortex_feature, points, trace=False)
    return out


# revision 40
# speedup vs baseline: 1.0305x; 1.0305x over previous
"""Gaussian falloff vortex-velocity kernel for Trainium2 (Bass/Tile).

Math per batch element b (single vortex y,x,tau,sig per batch):
    d1 = py - y;  d2 = px - x;  q = d1^2 + d2^2
    s  = tau * exp(-q/sig^2) / sqrt(q)
    out[..., 0] = s * d2;  out[..., 1] = -s * d1

16-bit formulation. The host quantizes the points with a per-batch
AFFINE map whose zero-point is the vortex and whose scale is
g = sqrt(2)/sig:  t1 = (y - py)*g,  t2 = (px - x)*g, rounded to bf16.
The zero-point kills catastrophic cancellation near the vortex (the
quantization error is relative to the distance d), and the scale
makes the on-chip math scalar-free:

    q' = t1^2 + t2^2  = 2 q / sig^2
    z' = q' + Ln(q' + 1e-30)
    s' = Exp(-z'/2 + ln tau)  = tau * exp(-q/sig^2) * sig/sqrt(2 q)
    out = s' * (t1, t2)       -- the g factors cancel exactly, giving
                                 the true velocity (v = s*d is
                                 invariant under scaling of d).

The eps in the Ln keeps s' finite at q'==0 so out = 0 * s' = 0; fp16
overflow of z' (q' > 65504 for tiny sig) gives s' = exp(-inf) = 0,
which matches the true falloff there. Outputs are fp16 (|v| <= tau
< 1), upcast on host. Simulated error vs the f32 reference:
l2 = 3.0e-3, scale-relative absmax = 7.9e-3 (gate: 2e-2).

Per core: 8 batches, each point-plane [512,512] -> [128, 2048].
Engine assignment (GpSimd deliberately unused -- it shares an SBUF
port with the vector engine and throttles concurrent DVE tensor ops
~4x, measured; Pool only drives the store-DMA descriptor ring, which
does not touch the SBUF data path):

    Qe = t1*t1            DVE tt bf16 (2x 16-bit mode, ~0.59ns/col)
    Qo = t2*t2            ACT Square (1x, any dtype)
    q' = Qe + Qo          DVE tt
    L  = Ln(q'+eps)       ACT fp16
    z' = q' + L           DVE tt fp16
    s' = Exp(-z'/2+lnt)   ACT bf16, per-batch bias AP
    O  = T * [s'|s']      DVE tt, ONE merged 4096-col op: s' rides a
                          stride-0 broadcast AP; plane order follows T
                          ([t1|t2]) so O = [v1|v0], unpacked swapped on
                          the host. (Merged 4096-col tts hold the 2x
                          mode; an earlier 1x reading was GpSimd
                          contention, not a size limit.)

Per batch: DVE 1223+1217+1217+2284 = 5.94us, ACT 3*1989 = 5.97us --
balanced. DMA 16MB/core at ~400GB/s = 40us, fully hidden. The 8-deep
software pipeline gives every cross-engine edge >= 1 full step of
slack, so no engine head-of-line blocks (each stage's inputs finished
during the previous step).
"""

import numpy as np
import ml_dtypes

import concourse.bass as bass
import concourse.bacc as bacc
import concourse.mybir as mybir
from concourse.tile import TileContext
from concourse.bass_utils import run_bass_kernel_spmd
from concourse.hw_specs import get_activation_tables

N_CORES = 8
B_PER_CORE = 8          # 64 batches / 8 cores
P = 128                 # SBUF partitions
W = 2048                # per-plane columns per partition (512*512/128)

BF16 = ml_dtypes.bfloat16

_PROGRAM = None


def _pin_act_table_set(arch: str):
    """Make all our activation functions resolve to the single
    `natural_log_exp_and_others` table set. The table-load inserter picks
    the FIRST set containing each function (Exp -> exp_and_others,
    Ln -> natural_log), which thrashes 2 table loads (~2.6us) per batch.
    get_activation_tables() is functools.cached and returns a mutable
    dict of sets; removing our functions from every other set (keeping
    indices intact) makes the combined set the unique first match."""
    AF = mybir.ActivationFunctionType
    try:
        tables = get_activation_tables(arch)
        keep = "natural_log_exp_and_others"
        needed = {AF.Identity, AF.Ln, AF.Exp, AF.Square, AF.Copy}
        if keep not in tables or not needed <= tables[keep]:
            return  # unexpected table layout: skip pinning (correct, slower)
        for name, fns in tables.items():
            if name != keep:
                fns -= needed
    except Exception:
        pass


def _build_program():
    f32 = mybir.dt.float32
    f16 = mybir.dt.float16
    bf16 = mybir.dt.bfloat16
    AF = mybir.ActivationFunctionType
    OP = mybir.AluOpType

    nc = bacc.Bacc(
        "TRN2",
        target_bir_lowering=False,
        debug=False,
        num_devices=N_CORES,
    )
    _pin_act_table_set(nc.m.arch)
    # tin cols: [t1-plane | t2-plane]; tout cols: [v1-plane | v0-plane]
    tin = nc.declare_dram_parameter("tin", [B_PER_CORE * P, 2 * W], bf16, isOutput=False)
    # consts: ln(tau) per batch + a trailing eps (1e-30) column for Ln's bias
    cst = nc.declare_dram_parameter("consts", [P, B_PER_CORE + 1], f32, isOutput=False)
    out = nc.declare_dram_parameter("tout", [B_PER_CORE * P, 2 * W], f16, isOutput=True)

    with TileContext(nc) as tc:
        with (
            tc.tile_pool(name="cpool", bufs=1) as cpool,
            tc.tile_pool(name="tp", bufs=8) as tp,        # T tiles (in planes)
            tc.tile_pool(name="qb", bufs=3) as qb_pool,   # squares
            tc.tile_pool(name="qq", bufs=4) as qq_pool,   # q'
            tc.tile_pool(name="lp", bufs=3) as l_pool,    # L
            tc.tile_pool(name="zp", bufs=3) as z_pool,    # z'
            tc.tile_pool(name="sp", bufs=3) as s_pool,    # s'
            tc.tile_pool(name="op", bufs=3) as o_pool,    # out planes
        ):
            # Consts ride the gpsimd DGE ring so the first T load is the
            # very first transfer on the sync ring (consts aren't needed
            # until the first Ln, ~4 steps in).
            c = cpool.tile([P, B_PER_CORE + 1], f32)
            nc.gpsimd.dma_start(c[:], cst[:])
            eps_ap = c[:, B_PER_CORE : B_PER_CORE + 1]

            # No warm-up activation: walrus hangs the ACT table load on the
            # first Square, which executes during that op's DMA wait -- off
            # the critical path for free.

            def cap(b):
                return c[:, b : b + 1]

            # 8-stage software pipeline over work items (batch col-chunks):
            #   A (step i):   load T(i)                       [SP ring]
            #   B (step i+1): Qe = t1^2 (DVE); Qo = t2^2 (ACT Square)
            #   C (step i+2): q' = Qe + Qo                    [DVE]
            #   D (step i+3): L = Ln(q'+eps)                  [ACT]
            #   E (step i+4): z' = q' + L                     [DVE]
            #   F (step i+5): s' = Exp(-z'/2 + ln tau)        [ACT]
            #   G (step i+6): O = T * [s'|s']  (merged)       [DVE]
            #   H (step i+7): store O                         [Pool DGE ring]
            # Every cross-engine dependency is at least one full step old, so
            # no engine's stream ever head-of-line blocks on in-flight work.
            # First/last batches split in col-halves to shorten fill/drain.
            items = []
            for b in range(B_PER_CORE):
                if b == 0:
                    # Graduated lead-in: a small first item gets the pipeline
                    # computing as soon as possible after DMA spin-up.
                    items.append((b, 0, W // 8))
                    items.append((b, W // 8, 3 * W // 8))
                    items.append((b, W // 2, W // 2))
                elif b == B_PER_CORE - 1:
                    items.append((b, 0, W // 2))
                    items.append((b, W // 2, W // 2))
                else:
                    items.append((b, 0, W))
            Ts, Qbs, qs, Ls, zs, ss, Os = {}, {}, {}, {}, {}, {}, {}

            def stage_a(i):
                b, c0, w = items[i]
                rows = slice(b * P, (b + 1) * P)
                T = tp.tile([P, 2 * w], bf16, tag="T")
                if w == W:
                    nc.sync.dma_start(T[:], tin[rows, :])
                else:
                    nc.sync.dma_start(T[:, :w], tin[rows, c0 : c0 + w])
                    nc.sync.dma_start(T[:, w:], tin[rows, W + c0 : W + c0 + w])
                Ts[i] = T

            def stage_b(i):
                b, c0, w = items[i]
                T = Ts[i]
                # DVE takes the t1 plane plus a small slice of t2 (one merged
                # tt, still 2x) so ACT runs ~170ns/item ahead of DVE and
                # phase wobble never stalls the vector engine. (k=0, the
                # nominal balance point, measures ~1.8us WORSE: dead-even
                # engines mutually stall.)
                k = w // 16
                Qb = qb_pool.tile([P, 2 * w], bf16, tag="Qb")
                nc.vector.tensor_tensor(Qb[:, : w + k], T[:, : w + k], T[:, : w + k], OP.mult)
                nc.scalar.activation(Qb[:, w + k :], T[:, w + k :], AF.Square)
                Qbs[i] = Qb

            def stage_c(i):
                b, c0, w = items[i]
                Qb = Qbs[i]
                q = qq_pool.tile([P, w], bf16, tag="q")
                nc.vector.tensor_tensor(q[:], Qb[:, :w], Qb[:, w:], OP.add)
                qs[i] = q

            def stage_d(i):
                b, c0, w = items[i]
                L = l_pool.tile([P, w], f16, tag="L")
                nc.scalar.activation(L[:], qs[i][:], AF.Ln, bias=eps_ap)
                Ls[i] = L

            def stage_e(i):
                b, c0, w = items[i]
                z = z_pool.tile([P, w], f16, tag="z")
                nc.vector.tensor_tensor(z[:], qs[i][:], Ls[i][:], OP.add)
                zs[i] = z

            def stage_f(i):
                b, c0, w = items[i]
                s = s_pool.tile([P, w], bf16, tag="s")
                nc.scalar.activation(s[:], zs[i][:], AF.Exp, bias=cap(b), scale=-0.5)
                ss[i] = s

            def stage_g(i):
                b, c0, w = items[i]
                T, s = Ts[i], ss[i]
                O = o_pool.tile([P, 2 * w], f16, tag="O")
                Ov = O.rearrange("p (c w) -> p c w", c=2)
                Tv = T.rearrange("p (c w) -> p c w", c=2)
                s_bc = s.rearrange("p (c w) -> p c w", c=1)[:].to_broadcast((P, 2, w))
                nc.vector.tensor_tensor(Ov[:], Tv[:], s_bc, OP.mult)
                Os[i] = O
                del Ts[i], Qbs[i], qs[i], Ls[i], zs[i], ss[i]

            def stage_h(i):
                b, c0, w = items[i]
                rows = slice(b * P, (b + 1) * P)
                O = Os[i]
                # Store triggers ride the GpSimd DGE ring: descriptor
                # generation does not touch the SBUF data path (unlike
                # gpsimd COMPUTE, which contends with DVE), and a separate
                # ring keeps store triggers from delaying loads.
                if w == W:
                    nc.gpsimd.dma_start(out[rows, :], O[:])
                else:
                    nc.gpsimd.dma_start(out[rows, c0 : c0 + w], O[:, :w])
                    nc.gpsimd.dma_start(out[rows, W + c0 : W + c0 + w], O[:, w:])
                del Os[i]

            # Within each step, the DVE stream runs [Qb, q', O, z']: the
            # O-product's input (s', 2 steps older) is safer than z's L, so
            # emitting O before z' gives the Ln->z' edge maximal slack.
            NI = len(items)
            for t in range(NI + 7):
                if t < NI:
                    stage_a(t)
                if 1 <= t < NI + 1:
                    stage_b(t - 1)
                if 2 <= t < NI + 2:
                    stage_c(t - 2)
                if 3 <= t < NI + 3:
                    stage_d(t - 3)
                if 6 <= t < NI + 6:
                    stage_g(t - 6)
                if 4 <= t < NI + 4:
                    stage_e(t - 4)
                if 5 <= t < NI + 5:
                    stage_f(t - 5)
                if t >= 7:
                    stage_h(t - 7)

    nc.compile()
    return nc


def _get_program():
    global _PROGRAM
    if _PROGRAM is None:
        _PROGRAM = _build_program()
    return _PROGRAM


def _make_in_maps(vortex_feature, points):
    B = points.shape[0]
    vf = np.asarray(vortex_feature, dtype=np.float32).reshape(B, 6)
    y, x, tau, sig = vf[:, 0], vf[:, 1], vf[:, 2], vf[:, 3]
    # Affine quantization scale g = sqrt(2)/sig. The sig floor keeps g (and
    # t*g) finite; for sig that small the true falloff underflows to 0, and
    # on-chip z' overflows fp16 -> s' = exp(-inf) = 0, matching it.
    g = (np.sqrt(2.0, dtype=np.float32) / np.maximum(sig, 1e-6)).astype(np.float32)
    with np.errstate(divide="ignore"):
        lnt = np.log(tau).astype(np.float32)  # tau==0 -> -inf -> s'=0
    ncol = B_PER_CORE + 1

    pts = np.asarray(points, dtype=np.float32)
    # Zero-point at the vortex (no cancellation near it), then scale by g.
    # t1 is negated (y - py) so out[...,1] = s' * t1.
    t1 = ((y[:, None, None] - pts[..., 0]) * g[:, None, None]).astype(BF16)
    t2 = ((pts[..., 1] - x[:, None, None]) * g[:, None, None]).astype(BF16)

    in_maps = []
    for i in range(N_CORES):
        sl = slice(i * B_PER_CORE, (i + 1) * B_PER_CORE)
        tin = np.concatenate(
            [t1[sl].reshape(B_PER_CORE * P, W), t2[sl].reshape(B_PER_CORE * P, W)],
            axis=1,
        )
        crow = np.concatenate([lnt[sl], np.float32([1e-30])]).reshape(1, ncol)
        cshard = np.ascontiguousarray(np.broadcast_to(crow, (P, ncol)))
        in_maps.append({"tin": np.ascontiguousarray(tin), "consts": cshard})
    return in_maps


def run(vortex_feature, points, trace=False, tmpdir=None):
    nc = _get_program()
    in_maps = _make_in_maps(vortex_feature, points)
    # The first execution of a freshly-loaded NEFF occasionally hits a
    # transient NRT_EXEC_UNIT_UNRECOVERABLE; a retry reliably succeeds.
    last_err = None
    for _ in range(3):
        try:
            res = run_bass_kernel_spmd(nc, in_maps, list(range(N_CORES)), trace=trace, tmpdir=tmpdir)
            break
        except Exception as err:  # noqa: BLE001
            last_err = err
    else:
        raise last_err
    B, H, Wd, _ = points.shape
    out = np.empty((B, H, Wd, 2), dtype=np.float32)
    for i in range(N_CORES):
        sl = slice(i * B_PER_CORE, (i + 1) * B_PER_CORE)
        o = res.results[i]["tout"].astype(np.float32)
        # Device plane order follows tin ([t1|t2]), so plane 0 = s'*t1 =
        # out[...,1] and plane 1 = s'*t2 = out[...,0].
        out[sl, ..., 1] = o[:, :W].reshape(B_PER_CORE, H, Wd)
        out[sl, ..., 0] = o[:, W:].reshape(B_PER_CORE, H, Wd)
    return out, res


def kernel(vortex_feature: np.ndarray, points: np.ndarray) -> np.ndarray:
    out, _ = run(vortex_feature, points, trace=False)
    return out
